# revision 14
# baseline (speedup 1.0000x reference)
"""CNNSelfAttention Trainium2 kernel (8 NeuronCores, SPMD).

Reference op: x (4,8,8,64,16,16); q,k,v = 3x3 convs 64->512ch (8 heads x
64ch); per-head attention over T=64 tile-images with head-dim 64*16*16;
unify 3x3 conv 512->64ch.  Output (4,8,8,64,16,16), all fp32.

Sharding: core c = 2*b + g handles batch b and heads 4g..4g+3.  Each core
computes q,k,v convs and attention for its 4 heads over all 64 images of
its batch, then a *partial* unify conv over its heads' 256 input
channels; a pairwise ReduceScatter ([0,1],[2,3],...) sums the two
partials of each batch and leaves each core with half the images, which
it writes out.  The host gathers the 8 half-batches.

Conv-as-matmul: every 3x3 conv is 9 shifted reads.  The host ships 3
dx-shifted, row-padded (18 rows x 16 cols), zero-filled copies of each
image, so each shifted read is one contiguous 256-element window
(offset dy*16).  SBUF partitions hold (channel, 2 row-copies): the upper
64 partitions hold the same data shifted one row, packing (dy, dy+1)
pairs into K=128 matmuls; dy=2 "singles" are zero-padded to K=128.

q,k use the *patches-stationary* (transposed) form so outputs land
pixels-on-partitions (the layout scores need); v and unify use the
weights-stationary form.  Convs run in fp32r (FP22 multiplies, fp32
accumulate); scores/apply/unify run in fp16 inputs with fp32 accumulate.
"""

import os
from contextlib import ExitStack

import numpy as np

F32 = None  # set in _lazy_imports
_BASS = {}


def _lazy_imports():
    if _BASS:
        return _BASS
    import concourse.bass as bass
    import concourse.tile as tile
    from concourse import bacc, mybir
    from concourse.bass_utils import run_bass_kernel_spmd
    from concourse.masks import make_identity

    _BASS.update(bass=bass, tile=tile, bacc=bacc, mybir=mybir,
                 run_bass_kernel_spmd=run_bass_kernel_spmd,
                 make_identity=make_identity)
    return _BASS


N_CORES = 8
B, NT, C, H, W = 4, 8, 8, 64, 16  # careful: reference is B,NT,NT,C,H,W
HH, WW = 16, 16
T = 64                   # images per batch
NH = 8                   # total heads
NH_LOC = 4               # heads per core
RL = 18 * 16             # row-padded image length (x3 layout)
AL = 16 * 18             # col-padded attn/v image length
CHUNK_IMGS = 2
N_APPLY_CHUNKS = (64 * AL) // 512  # 36

_CACHE = {}


def _build():
    if "nc" in _CACHE:
        return _CACHE["nc"]
    m = _lazy_imports()
    bass, tile, bacc, mybir = m["bass"], m["tile"], m["bacc"], m["mybir"]
    make_identity = m["make_identity"]
    F32 = mybir.dt.float32
    F32R = mybir.dt.float32r
    F16 = mybir.dt.float16
    CC = 64  # channels

    nc = bacc.Bacc("TRN2", target_bir_lowering=False, debug=False,
                   num_devices=N_CORES)

    x3 = nc.dram_tensor("x3", [3, CC, T * RL], F32R, kind="ExternalInput").ap()
    wqk = nc.dram_tensor("wqk", [128, 6, 512], F32R, kind="ExternalInput").ap()
    wv = nc.dram_tensor("wv", [128, 6, 256], F32R, kind="ExternalInput").ap()
    wu = nc.dram_tensor("wu", [128, NH_LOC, 9, 64], F16, kind="ExternalInput").ap()
    out = nc.dram_tensor("out", [T // 2, CC * HH * WW], F32, kind="ExternalOutput").ap()

    with tile.TileContext(nc) as tc, ExitStack() as top:
        perm = top.enter_context(tc.tile_pool(name="perm", bufs=1))
        dram = top.enter_context(tc.tile_pool(name="dram", bufs=1, space="DRAM"))
        # vh0 outlives qkT (LIFO pool stack): allocate it first
        vh_pool0 = tc.alloc_tile_pool(name="vh0", bufs=1)
        qkT_pool = tc.alloc_tile_pool(name="qkTp", bufs=1)
        wpool = tc.alloc_tile_pool(name="wpool", bufs=1)

        wqk_t = wpool.tile([128, 6, 512], F32R)
        nc.sync.dma_start(wqk_t[:], wqk[:])
        wv_t = wpool.tile([128, 6, 256], F32R)
        nc.sync.dma_start(wv_t[:], wv[:])
        wu_t = perm.tile([128, NH_LOC, 9, 64], F16)
        nc.sync.dma_start(wu_t[:], wu[:])
        ident = perm.tile([64, 64], F16)
        make_identity(nc, ident[:])

        # pixels on partitions; free = (qk, h, c, pixchunk, img)
        qkT = qkT_pool.tile([128, 2, NH_LOC, CC, 2, T], F16)

        v_dram = dram.tile([T, NH_LOC, CC * AL], F16)
        attn_dram = dram.tile([NH_LOC, CC, T, AL], F16)
        rs_in = dram.tile([T, CC * HH * WW], F32)
        rs_out = dram.tile([T // 2, CC * HH * WW], F32)

        KGROUPS = [(0, 0, 0), (1, 0, 1), (2, 0, 2),
                   (0, 32, 3), (1, 32, 4), (2, 32, 5)]

        # ---------------- Phase A: q,k,v convs ----------------
        with tc.tile_pool(name="xpool", bufs=2) as xpool, \
             tc.tile_pool(name="qkps", bufs=4, space="PSUM") as qkps, \
             tc.tile_pool(name="vps", bufs=2, space="PSUM") as vps, \
             tc.tile_pool(name="vstage", bufs=2) as vstage_pool:
            CL = CHUNK_IMGS * RL
            for ch in range(T // CHUNK_IMGS):
                i0 = ch * CHUNK_IMGS
                xts = []
                for dx in range(3):
                    xt = xpool.tile([128, CL + 16], F32R, tag=f"x{dx}", name=f"xt{dx}")
                    nc.gpsimd.memset(xt[:, CL - 16:].bitcast(F32), 0.0)
                    nc.sync.dma_start(xt[0:64, 0:CL],
                                      x3[dx, :, i0 * RL:(i0 + CHUNK_IMGS) * RL])
                    nc.sync.dma_start(xt[64:128, 0:CL - 16],
                                      x3[dx, :, i0 * RL + 16:(i0 + CHUNK_IMGS) * RL])
                    xts.append(xt)

                for il in range(CHUNK_IMGS):
                    for pc in range(2):
                        acc = qkps.tile([128, 512], F32, tag="qkacc", name="qkacc")
                        for g, (dx, off, wg) in enumerate(KGROUPS):
                            base = il * RL + off + pc * 128
                            nc.tensor.matmul(acc[:], xts[dx][:, base:base + 128],
                                             wqk_t[:, wg, :],
                                             start=(g == 0), stop=(g == 5))
                        dst = qkT[:, :, :, :, pc, i0 + il]
                        nc.vector.tensor_copy(
                            dst, acc[:].rearrange("p (a h c) -> p a h c", a=2, h=NH_LOC))

                for pl in range(CHUNK_IMGS // 2):
                    ip = i0 + 2 * pl
                    stgs = []
                    for mc in range(2):
                        acc = vps.tile([128, 512], F32, tag="vacc", name="vacc")
                        for g, (dx, off, wg) in enumerate(KGROUPS):
                            xv = xts[dx][:, 0:CL].rearrange("p (i e) -> p i e",
                                                            i=CHUNK_IMGS)
                            rhs = xv[:, 2 * pl:2 * pl + 2, off:off + 256]
                            nc.tensor.matmul(acc[:], wv_t[:, wg, bass.ts(mc, 128)],
                                             rhs, start=(g == 0), stop=(g == 5))
                        st = vstage_pool.tile([128, 2, 16, 18], F16,
                                              tag=f"vst{mc}", name=f"vst{mc}")
                        nc.gpsimd.memset(st[:, :, :, 0:18:17], 0.0)
                        nc.scalar.copy(
                            st[:, :, :, 1:17],
                            acc[:].rearrange("p (i y x) -> p i y x", i=2, y=16))
                        stgs.append(st)
                    # spill: v_dram[s][(h,c)][e]
                    vd = v_dram[:].rearrange("s h (c e) -> (h c) s e", c=CC)
                    for mc in range(2):
                        nc.sync.dma_start(
                            vd[mc * 128:(mc + 1) * 128, ip:ip + 2, :],
                            stgs[mc][:].rearrange("p i y x -> p i (y x)"))

        # weights for convs are no longer needed after phase A
        wpool.release()

        # ---------------- Phase B: v-hat prefetch, scores, softmax, w^T ----
        wts = []
        with tc.tile_pool(name="sps", bufs=2, space="PSUM") as sps, \
             tc.tile_pool(name="smax", bufs=1) as smax, \
             tc.tile_pool(name="tps", bufs=2, space="PSUM") as tps:
            # only pair 0 fits alongside qkT; pair 1 loads at phase C start
            vhp = [vh_pool0.tile([128, CC * AL], F16, name="vhp0")]
            nc.sync.dma_start(vhp[0][0:64, :], v_dram[:, 0, :])
            nc.sync.dma_start(vhp[0][64:128, :], v_dram[:, 1, :])

            s_sb = smax.tile([64, NH_LOC, T], F32)
            for h in range(NH_LOC):
                acc = sps.tile([64, 64], F32, tag=f"sacc{h % 2}", name="sacc")
                first = True
                for c in range(CC):
                    for pc in range(2):
                        nc.tensor.matmul(acc[:], qkT[:, 0, h, c, pc, :],
                                         qkT[:, 1, h, c, pc, :], start=first,
                                         stop=(c == CC - 1 and pc == 1))
                        first = False
                nc.vector.tensor_copy(s_sb[:, h, :], acc[:])

            mybir = _BASS["mybir"]
            nmax = smax.tile([64, NH_LOC, 1], F32)
            nc.vector.tensor_reduce(nmax[:], s_sb[:], axis=mybir.AxisListType.X,
                                    op=mybir.AluOpType.max, negate=True)
            eterm = smax.tile([64, NH_LOC, T], F32)
            nc.vector.tensor_tensor(out=eterm[:], in0=s_sb[:],
                                    in1=nmax[:].broadcast_to((64, NH_LOC, T)),
                                    op=mybir.AluOpType.add)
            nc.scalar.activation(eterm[:], eterm[:],
                                 mybir.ActivationFunctionType.Exp)
            esum = smax.tile([64, NH_LOC, 1], F32)
            nc.vector.tensor_reduce(esum[:], eterm[:], axis=mybir.AxisListType.X,
                                    op=mybir.AluOpType.add)
            einv = smax.tile([64, NH_LOC, 1], F32)
            nc.vector.reciprocal(einv[:], esum[:])
            w_sb = smax.tile([64, NH_LOC, T], F16)
            nc.vector.tensor_tensor(out=w_sb[:], in0=eterm[:],
                                    in1=einv[:].broadcast_to((64, NH_LOC, T)),
                                    op=mybir.AluOpType.mult)

            for j in range(2):
                tp = tps.tile([128, 64], F16, tag="wtp", name="wtp")
                nc.tensor.transpose(
                    tp[:], w_sb[:, 2 * j:2 * j + 2, :].rearrange("p a b -> p (a b)"),
                    ident[:])
                wt = perm.tile([128, 64], F16, name=f"wt{j}")
                nc.vector.tensor_copy(wt[:], tp[:])
                wts.append(wt)

        # qkT is dead after the score matmuls; free its space for vhp[1]
        qkT_pool.release()

        # ---------------- Phase C: apply + attn spill ----------------
        with tc.tile_pool(name="vh1", bufs=1) as vh_pool1, \
             tc.tile_pool(name="astage", bufs=2) as astage_pool, \
             tc.tile_pool(name="aps", bufs=4, space="PSUM") as aps:
            vt = vh_pool1.tile([128, CC * AL], F16, name="vhp1")
            nc.sync.dma_start(vt[0:64, :], v_dram[:, 2, :])
            nc.sync.dma_start(vt[64:128, :], v_dram[:, 3, :])
            vhp.append(vt)
            for h in range(NH_LOC):
                j, r = divmod(h, 2)
                stg = astage_pool.tile([64, 64 * AL], F16, tag="ast", name="ast")
                psl = slice(64 * r, 64 * (r + 1))
                for chk in range(N_APPLY_CHUNKS):
                    acc = aps.tile([64, 512], F32, tag="aacc", name="aacc")
                    nc.tensor.matmul(acc[:], wts[j][psl, :],
                                     vhp[j][psl, chk * 512:(chk + 1) * 512],
                                     start=True, stop=True)
                    if chk % 2 == 0:
                        nc.vector.tensor_copy(stg[:, chk * 512:(chk + 1) * 512], acc[:])
                    else:
                        nc.scalar.copy(stg[:, chk * 512:(chk + 1) * 512], acc[:])
                nc.sync.dma_start(attn_dram[h].transpose([1, 0, 2]),
                                  stg[:].rearrange("p (c e) -> p c e", c=CC))
        vh_pool0.release()

        # ---------------- Phase D: unify conv (partial over local heads) ---
        with tc.tile_pool(name="upatch", bufs=1) as upatch, \
             tc.tile_pool(name="ups", bufs=4, space="PSUM") as ups, \
             tc.tile_pool(name="ustage", bufs=2) as ustage:
            patches = []
            for h in range(NH_LOC):
                pt = upatch.tile([128, T * AL], F16, tag=f"pat{h}", name=f"pat{h}")
                nc.gpsimd.memset(pt[64:128, T * AL - 18:], 0.0)
                af = attn_dram[h].rearrange("c t e -> c (t e)")
                nc.sync.dma_start(pt[0:64, :], af)
                nc.sync.dma_start(pt[64:128, 0:T * AL - 18], af[:, 18:])
                patches.append(pt)

            rs_iv = rs_in[:].rearrange("s (c e) -> c s e", c=CC)
            for pl in range(T // 2):
                ip = 2 * pl
                acc = ups.tile([64, 2, 256], F32, tag="uacc", name="uacc")
                first = True
                for h in range(NH_LOC):
                    pv = patches[h][:].rearrange("p (t y x) -> p t y x", t=T, y=16)
                    for g in range(9):
                        dx = g % 3
                        kind = g // 3
                        if kind == 0:      # pairs (dy=1, dy=2): out y 0..14
                            outv = acc[:, :, 0:240]
                            rhs = pv[:, ip:ip + 2, 0:15, dx:dx + 16]
                        elif kind == 1:    # singles (dy=0): out y 1..15
                            outv = acc[:, :, 16:256]
                            rhs = pv[:, ip:ip + 2, 0:15, dx:dx + 16]
                        else:              # fixups (dy=1 @ y15): out y 15
                            outv = acc[:, :, 240:256]
                            rhs = pv[:, ip:ip + 2, 15:16, dx:dx + 16]
                        nc.tensor.matmul(outv, wu_t[:, h, g, :], rhs,
                                         start=first,
                                         stop=(h == NH_LOC - 1 and g == 8))
                        first = False
                st = ustage.tile([64, 2, 256], F32, tag="ust", name="ust")
                nc.scalar.copy(st[:], acc[:])
                nc.sync.dma_start(rs_iv[:, ip:ip + 2, :], st[:])

        # ---------------- Phase E: pairwise ReduceScatter + output --------
        nc.gpsimd.collective_compute(
            "ReduceScatter", _BASS["mybir"].AluOpType.add,
            replica_groups=[[0, 1], [2, 3], [4, 5], [6, 7]],
            ins=[rs_in[:].opt()], outs=[rs_out[:].opt()])
        nc.sync.dma_start(out[:], rs_out[:])

    nc.compile()
    _CACHE["nc"] = nc
    return nc


# ======================= host-side packing =======================

def _pack_inputs(x, Wq, Wk, Wv, Wu):
    """Build the 8 per-core input maps (all numpy)."""
    x = np.asarray(x, np.float32)
    Wq = np.asarray(Wq, np.float32)
    Wk = np.asarray(Wk, np.float32)
    Wv = np.asarray(Wv, np.float32)
    Wu = np.asarray(Wu, np.float32)
    scale = float((HH * WW) ** 0.25)
    Wqs = Wq / scale
    Wks = Wk / scale

    xf = x.reshape(B, T, 64, HH, WW)

    # x3 per batch: (3, 64, T, 18, 16)
    x3_all = np.zeros((B, 3, 64, T, 18, 16), np.float32)
    for dx in range(3):
        sh = dx - 1
        src = xf.transpose(0, 2, 1, 3, 4)  # (B, c, T, y, x)
        if sh == -1:
            x3_all[:, dx, :, :, 1:17, 1:16] = src[:, :, :, :, 0:15]
        elif sh == 0:
            x3_all[:, dx, :, :, 1:17, :] = src
        else:
            x3_all[:, dx, :, :, 1:17, 0:15] = src[:, :, :, :, 1:16]
    x3_all = x3_all.reshape(B, 3, 64, T * RL)

    in_maps = []
    for core in range(N_CORES):
        b, g = divmod(core, 2)
        hs = [g * NH_LOC + h for h in range(NH_LOC)]  # global head ids
        och = np.concatenate([np.arange(h * 64, (h + 1) * 64) for h in hs])

        # wqk: (128, 6, 512): N = [q 4h*64 | k 4h*64]
        wqk_np = np.zeros((128, 6, 512), np.float32)
        for grp in range(3):
            dx = grp
            wqk_np[0:64, grp, 0:256] = Wqs[och, :, 0, dx].T
            wqk_np[64:128, grp, 0:256] = Wqs[och, :, 1, dx].T
            wqk_np[0:64, grp, 256:512] = Wks[och, :, 0, dx].T
            wqk_np[64:128, grp, 256:512] = Wks[och, :, 1, dx].T
        for dx in range(3):
            wqk_np[0:64, 3 + dx, 0:256] = Wqs[och, :, 2, dx].T
            wqk_np[0:64, 3 + dx, 256:512] = Wks[och, :, 2, dx].T

        # wv: (128, 6, 256): M-chunks of 128 out-ch
        wv_np = np.zeros((128, 6, 256), np.float32)
        for grp in range(3):
            dx = grp
            wv_np[0:64, grp, :] = Wv[och, :, 0, dx].T
            wv_np[64:128, grp, :] = Wv[och, :, 1, dx].T
        for dx in range(3):
            wv_np[0:64, 3 + dx, :] = Wv[och, :, 2, dx].T

        # wu: (128, 4, 9, 64): per local head, groups:
        #   g0-2 pairs:   rows0-63 = Wu[:, hc, dy=1, dx].T ; rows64-127 = dy=2
        #   g3-5 singles: rows0-63 = dy=0 ; upper 0
        #   g6-8 fixups:  rows0-63 = dy=1 ; upper 0
        wu_np = np.zeros((128, NH_LOC, 9, 64), np.float32)
        for hl, hg in enumerate(hs):
            ic = np.arange(hg * 64, (hg + 1) * 64)
            for dx in range(3):
                wu_np[0:64, hl, dx, :] = Wu[:, ic, 1, dx].T
                wu_np[64:128, hl, dx, :] = Wu[:, ic, 2, dx].T
                wu_np[0:64, hl, 3 + dx, :] = Wu[:, ic, 0, dx].T
                wu_np[0:64, hl, 6 + dx, :] = Wu[:, ic, 1, dx].T

        in_maps.append({
            "x3": np.ascontiguousarray(x3_all[b]),
            "wqk": wqk_np,
            "wv": wv_np,
            "wu": wu_np.astype(np.float16),
        })
    return in_maps


def _assemble(results):
    """results[core]['out'] (32, 64*256) -> (4,8,8,64,16,16)."""
    full = np.zeros((B, T, 64, HH, WW), np.float32)
    for core in range(N_CORES):
        b, g = divmod(core, 2)
        part = results[core]["out"].reshape(32, 64, HH, WW)
        full[b, g * 32:(g + 1) * 32] = part
    return full.reshape(B, NT, NT, 64, HH, WW)


def _get_runner():
    """Build (once) a sharded jitted executable for the 8-core NEFF.

    Returns (fn, in_names, out_names, out_shapes): fn takes concatenated
    per-core inputs (axis 0) and returns concatenated outputs.
    """
    if "runner" in _CACHE:
        return _CACHE["runner"]
    import jax
    from jax.sharding import Mesh, PartitionSpec
    from jax.experimental.shard_map import shard_map
    from concourse import bass2jax
    import concourse.mybir as mybir

    nc = _build()
    bass2jax.install_neuronx_cc_hook()
    partition_name = nc.partition_id_tensor.name if nc.partition_id_tensor else None
    in_names, out_names, out_avals, zero_outs = [], [], [], []
    for alloc in nc.m.functions[0].allocations:
        if not isinstance(alloc, mybir.MemoryLocationSet):
            continue
        name = alloc.memorylocations[0].name
        if alloc.kind == "ExternalInput":
            if name != partition_name:
                in_names.append(name)
        elif alloc.kind == "ExternalOutput":
            out_names.append(name)
            shape = tuple(alloc.tensor_shape)
            dtype = mybir.dt.np(alloc.dtype)
            out_avals.append(jax.core.ShapedArray(shape, dtype))
            zero_outs.append(np.zeros(shape, dtype))
    n_params = len(in_names)
    n_outs = len(out_avals)
    all_in_names = list(in_names) + list(out_names)
    if partition_name is not None:
        all_in_names.append(partition_name)

    def _body(*args):
        operands = list(args)
        if partition_name is not None:
            operands.append(bass2jax.partition_id_tensor())
        outs = bass2jax._bass_exec_p.bind(
            *operands, out_avals=tuple(out_avals), in_names=tuple(all_in_names),
            out_names=tuple(out_names), lowering_input_output_aliases=(),
            sim_require_finite=True, sim_require_nnan=True, nc=nc)
        return tuple(outs)

    import numpy as _np
    devices = jax.devices()[:N_CORES]
    mesh = Mesh(_np.asarray(devices), ("core",))
    in_specs = (PartitionSpec("core"),) * (n_params + n_outs)
    out_specs = (PartitionSpec("core"),) * n_outs
    fn = jax.jit(shard_map(_body, mesh=mesh, in_specs=in_specs,
                           out_specs=out_specs, check_rep=False),
                 keep_unused=True)
    concat_zeros = [np.zeros((N_CORES * z.shape[0], *z.shape[1:]), z.dtype)
                    for z in zero_outs]
    _CACHE["runner"] = (fn, in_names, out_names, out_avals, concat_zeros)
    return _CACHE["runner"]


def _run_maps(in_maps):
    import jax
    fn, in_names, out_names, out_avals, concat_zeros = _get_runner()
    concat_in = [np.concatenate([in_maps[c][nm] for c in range(N_CORES)], axis=0)
                 for nm in in_names]
    outs = fn(*concat_in, *concat_zeros)
    results = []
    for c in range(N_CORES):
        results.append({
            nm: np.asarray(outs[i]).reshape(N_CORES, *out_avals[i].shape)[c]
            for i, nm in enumerate(out_names)})
    return results


def kernel(x, Wq, Wk, Wv, Wu):
    _lazy_imports()
    in_maps = _pack_inputs(x, Wq, Wk, Wv, Wu)
    return _assemble(_run_maps(in_maps))


if __name__ == "__main__":
    nc = _build()
    print("build + compile OK")
